# revision 1
# baseline (speedup 1.0000x reference)
"""Trainium2 Bass kernel for CausalCoreV5 (complex-weight GNN message passing).

Math: reference does, per step t:
    theta = raw_phase + omega*t ;  c,s = cos(theta), sin(theta)
    Aamp  = A_mask * G_gate * tanh(raw_S) * sigmoid(raw_r)
    out_r = (Aamp*c)@xr - (Aamp*s)@xi ;  out_i = (Aamp*s)@xr + (Aamp*c)@xi
    x'    = tanh([out_r, out_i])

Angle-addition turns the time-varying matrices into two FIXED matrices:
    P = Aamp*cos(raw_phase), Q = Aamp*sin(raw_phase)
    u = P@xr - Q@xi ; v = Q@xr + P@xi
    out_r = cos(wt)*u - sin(wt)*v ; out_i = sin(wt)*u + cos(wt)*v

Each of 8 cores owns 512 output rows; P^T,Q^T slices (4096x512, f32 stored,
f32r matmuls) live in SBUF for all 32 steps. The per-step rotation is folded
into the stationary weights (x1 = [c*xr-s*xi, s*xr+c*xi], w2 = [-xi', xr']),
so a step is: 64 PE matvecs accumulating [u;v] -> tanh(PSUM) -> 8-core
AllGather of the [2,512] state slice -> PE-transpose the gathered state back
into weight layout. Dummy matmuls keep the PE HAM-warm across the AllGather.
"""

import os
import sys

import numpy as np

if "/opt/trn_rl_repo" not in sys.path:
    sys.path.insert(0, "/opt/trn_rl_repo")

N = 4096
STEPS = 32
NCORES = 8
ROWS = N // NCORES          # 512 output rows per core
KT = N // 128               # 32 contraction k-tiles of 128
N_WARM = 48                  # dummy warm-keeper matmuls per AllGather gap

_CACHE = {}


def _build_nc():
    import math

    from concourse import bacc, bass, masks, mybir, tile
    from concourse.bass import AP

    f32 = mybir.dt.float32
    f32r = mybir.dt.float32r
    AF = mybir.ActivationFunctionType
    HALF_PI = math.pi / 2.0

    nc = bacc.Bacc(
        "TRN2",
        target_bir_lowering=False,
        debug=False,
        enable_asserts=True,
        num_devices=NCORES,
    )

    # Register pi/2 as a const AP (used as Sin bias to get cos); mirrors the
    # 0.0/1.0 registration in the Bass constructor.
    _hp = nc.alloc_sbuf_tensor("const-halfpi", [128, 1], f32)
    nc.gpsimd.memset(_hp.ap(), HALF_PI)
    nc.const_aps.aps[(f32, HALF_PI)] = _hp.ap()
    nc.all_engine_barrier()

    # xfull comes in TRANSPOSED: [2, N] (xr row, xi row) for fast strided DMA.
    xfull = nc.dram_tensor("xfull", [2, N], f32, kind="ExternalInput")
    s_sl = nc.dram_tensor("s_sl", [ROWS, N], f32, kind="ExternalInput")
    ph_sl = nc.dram_tensor("ph_sl", [ROWS, N], f32, kind="ExternalInput")
    r_sl = nc.dram_tensor("r_sl", [ROWS, N], f32, kind="ExternalInput")
    m_sl = nc.dram_tensor("m_sl", [ROWS, N], f32, kind="ExternalInput")
    g_sl = nc.dram_tensor("g_sl", [ROWS, N], f32, kind="ExternalInput")
    # Per-step rotation scalars cos(wt), sin(wt) broadcast down 128 partitions
    # (cols 2t, 2t+1); derived on host from the scalar omega input.
    wrot = nc.dram_tensor("wrot", [128, 2 * STEPS], f32, kind="ExternalInput")
    # out is [steps+1, 2, N]; host transposes to [steps+1, N, 2].
    out = nc.dram_tensor("out", [STEPS + 1, 2, N], f32, kind="ExternalOutput")

    with tile.TileContext(nc) as tc:
        with (
            tc.tile_pool(name="big", bufs=1) as big,
            tc.tile_pool(name="work", bufs=2) as work,
            tc.tile_pool(name="small", bufs=2) as small,
            tc.tile_pool(name="psA", bufs=3, space="PSUM") as psA,
            tc.tile_pool(name="psB", bufs=2, space="PSUM") as psB,
            tc.tile_pool(name="dram", bufs=2, space="DRAM") as dpool,
        ):
            ident = big.tile([128, 128], f32, name="ident", tag="ident")
            masks.make_identity(nc, ident)

            # Persistent transposed matrices: PT[k, n], QT[k, n] stored as 32
            # k-tiles of [128, 512] side by side -> [128, 32*512].
            pt = big.tile([128, KT * 512], f32, name="pt", tag="pt")
            qt = big.tile([128, KT * 512], f32, name="qt", tag="qt")
            ptH = pt.tensor
            qtH = qt.tensor

            # ---------------- Phase A: load inputs, build P^T, Q^T ----------
            CC = 1024  # column chunk width
            for rb in range(ROWS // 128):  # 4 row blocks of 128
                for cc in range(N // CC):  # 4 column chunks of 1024
                    r0, r1 = rb * 128, (rb + 1) * 128
                    c0, c1 = cc * CC, (cc + 1) * CC
                    s_in = work.tile([128, CC], f32, name=f"s_{rb}_{cc}", tag="s_in")
                    r_in = work.tile([128, CC], f32, name=f"r_{rb}_{cc}", tag="r_in")
                    m_in = work.tile([128, CC], f32, name=f"m_{rb}_{cc}", tag="m_in")
                    g_in = work.tile([128, CC], f32, name=f"g_{rb}_{cc}", tag="g_in")
                    p_in = work.tile([128, CC], f32, name=f"p_{rb}_{cc}", tag="p_in")
                    nc.sync.dma_start(s_in, s_sl[r0:r1, c0:c1])
                    nc.sync.dma_start(r_in, r_sl[r0:r1, c0:c1])
                    nc.sync.dma_start(m_in, m_sl[r0:r1, c0:c1])
                    # spread onto SWDGE queues (PE sequencer is idle here) so
                    # HWDGE's 8 queues aren't the aggregate-bandwidth cap
                    nc.gpsimd.dma_start(g_in, g_sl[r0:r1, c0:c1])
                    nc.gpsimd.dma_start(p_in, ph_sl[r0:r1, c0:c1])

                    # amp = mask*gate*tanh(S)*sigmoid(r); sigmoid via tanh:
                    # sigmoid(x) = 0.5*tanh(0.5*x) + 0.5  (keeps ACT on one LUT)
                    # Alternate Tanh/Sin emission order per chunk parity so the
                    # ACT engine reloads its LUT once per chunk, not twice.
                    cos_t = work.tile([128, CC], f32, name=f"c_{rb}_{cc}", tag="cos_t")
                    sin_t = work.tile([128, CC], f32, name=f"n_{rb}_{cc}", tag="sin_t")

                    def _tanh_ops():
                        nc.scalar.activation(s_in, s_in, AF.Tanh)
                        nc.scalar.activation(r_in, r_in, AF.Tanh, scale=0.5)

                    def _sin_ops():
                        nc.scalar.activation(cos_t, p_in, AF.Sin, bias=HALF_PI)
                        nc.scalar.activation(sin_t, p_in, AF.Sin)

                    if (rb * (N // CC) + cc) % 2 == 0:
                        _tanh_ops(); _sin_ops()
                    else:
                        _sin_ops(); _tanh_ops()

                    nc.vector.tensor_scalar(
                        r_in, r_in, 0.5, 0.5,
                        op0=mybir.AluOpType.mult, op1=mybir.AluOpType.add,
                    )
                    nc.gpsimd.tensor_mul(m_in, m_in, g_in)
                    nc.gpsimd.tensor_mul(m_in, m_in, s_in)
                    nc.vector.tensor_mul(m_in, m_in, r_in)
                    nc.vector.tensor_mul(cos_t, cos_t, m_in)  # P chunk
                    nc.vector.tensor_mul(sin_t, sin_t, m_in)  # Q chunk

                    # Transpose each 128x128 sub-chunk via PE, land 4 at a time
                    # in one PSUM bank, then one strided copy into pt/qt.
                    for src, dstH, nm in ((cos_t, ptH, "p"), (sin_t, qtH, "q")):
                        for grp in range(CC // 512):  # 2 groups of 4 subchunks
                            ps = psA.tile(
                                [128, 512], f32,
                                name=f"tr_{nm}_{rb}_{cc}_{grp}", tag="tr",
                            )
                            for j in range(4):
                                sub = grp * 4 + j
                                nc.tensor.transpose(
                                    ps[:, j * 128:(j + 1) * 128],
                                    src[:, sub * 128:(sub + 1) * 128],
                                    ident,
                                )
                            kt0 = cc * (CC // 128) + grp * 4
                            dst = AP(
                                dstH, kt0 * 512 + rb * 128,
                                [[KT * 512, 128], [512, 4], [1, 128]],
                            )
                            srcp = AP(ps.tensor, 0, [[512, 128], [128, 4], [1, 128]])
                            nc.vector.tensor_copy(dst.bitcast(f32r), srcp)

            # ---------------- per-step rotation scalars ----------------------
            W = 2 * STEPS
            wrs = small.tile([128, W], f32, name="wrs", tag="wrs", bufs=1)
            nc.sync.dma_start(wrs, wrot[0:128, 0:W])
            wrsH = wrs.tensor

            # ---------------- initial state + out[0] -------------------------
            nc.sync.dma_start(
                AP(out, 0, [[N, 2], [1, N]]),
                AP(xfull, 0, [[N, 2], [1, N]]),
            )

            def load_xw(t, src_ap):
                """xa [16,512] (rank,comp major) -> x1 [128,64] per-ktile
                [xr|xi] weight cols and w2 = [-xi|xr], via PE transposes.

                xa[(r,c), nl] = x[c, r*512+nl];  x1[p, 2*kt+c] with
                kt = 4*r + j, nl = j*128 + p.
                """
                xa = work.tile([16, 512], f32, name=f"xa_{t}", tag="xa")
                if src_ap is None:
                    # xfull [2, N]: xa[(r,c), nl] <- addr c*N + r*512 + nl
                    src_ap = AP(xfull, 0, [[512, 8], [N, 2], [1, 512]])
                nc.sync.dma_start(xa, src_ap)
                x1 = work.tile([128, 2 * KT], f32, name=f"x1_{t}", tag="x1")
                w2 = work.tile([128, 2 * KT], f32, name=f"w2_{t}", tag="w2")
                x1H, w2H = x1.tensor, w2.tensor
                psx = psB.tile([128, 64], f32, name=f"px_{t}", tag="px")
                for j in range(4):
                    # psx[p, 16j + (2r+c)] = xa[(r,c), j*128+p]
                    nc.tensor.transpose(
                        psx[:, 16 * j:16 * (j + 1)],
                        xa[:, j * 128:(j + 1) * 128],
                        ident[0:16, 0:16],
                    )
                pxH = psx.tensor
                # Fold the step-t rotation into the weights:
                #   xr' = c*xr - s*xi ; xi' = s*xr + c*xi
                # x1 = [xr', xi'] per ktile, w2 = [-xi', xr'].
                c_t = AP(wrsH, 2 * t, [[W, 128], [1, 1]])
                s_t = AP(wrsH, 2 * t + 1, [[W, 128], [1, 1]])
                xr_ap = AP(pxH, 0, [[64, 128], [16, 4], [2, 8]])
                xi_ap = AP(pxH, 1, [[64, 128], [16, 4], [2, 8]])
                tA = small.tile([128, KT], f32, name=f"tA_{t}", tag="tA")
                tB = small.tile([128, KT], f32, name=f"tB_{t}", tag="tB")
                # tA/tB must be kt-ordered: kt = 4r + j for iter dims (j, r)
                t3 = [[KT, 128], [1, 4], [4, 8]]
                tC = small.tile([128, KT], f32, name=f"tC_{t}", tag="tC")
                tD = small.tile([128, KT], f32, name=f"tD_{t}", tag="tD")
                # Critical chain first: x1 even cols (xr') gate the PT matmuls.
                nc.vector.tensor_scalar_mul(AP(tA.tensor, 0, t3), xr_ap, c_t)
                nc.vector.tensor_scalar_mul(AP(tB.tensor, 0, t3), xi_ap, s_t)
                nc.vector.tensor_tensor(
                    AP(x1H, 0, [[2 * KT, 128], [2, KT]]).bitcast(f32r),
                    tA, tB, op=mybir.AluOpType.subtract,
                )
                # Lagging ops overlap the PT matmul burst (QT MMs run later).
                nc.vector.tensor_scalar_mul(AP(tC.tensor, 0, t3), xr_ap, s_t)
                nc.vector.tensor_scalar_mul(AP(tD.tensor, 0, t3), xi_ap, c_t)
                nc.vector.tensor_tensor(
                    AP(x1H, 1, [[2 * KT, 128], [2, KT]]).bitcast(f32r),
                    tC, tD, op=mybir.AluOpType.add,
                )
                nc.vector.tensor_scalar_mul(
                    AP(w2H, 0, [[2 * KT, 128], [2, KT]]).bitcast(f32r),
                    AP(x1H, 1, [[2 * KT, 128], [2, KT]]),
                    -1.0,
                )
                nc.vector.tensor_copy(
                    AP(w2H, 1, [[2 * KT, 128], [2, KT]]).bitcast(f32r),
                    AP(x1H, 0, [[2 * KT, 128], [2, KT]]),
                )
                return x1, w2

            # Cheap warm-keeper operands: bf16 garbage tiles, memset once.
    	    # (content never read back; keeps PE HAM-warm during AllGather)
            bf16 = mybir.dt.bfloat16
            wk_w = big.tile([128, 2], bf16, name="wk_w", tag="wk_w")
            wk_r = big.tile([128, 64], bf16, name="wk_r", tag="wk_r")
            nc.gpsimd.memset(wk_w, 0)
            nc.gpsimd.memset(wk_r, 0)

            # t=0 state comes from xfull (src_ap=None selects that path)
            x1, w2 = load_xw(0, None)

            for t in range(STEPS):
                psuv = psB.tile([2, 512], f32, name=f"uv_{t}", tag="uv")
                x1H, w2H = x1.tensor, w2.tensor
                for kt in range(KT):
                    nc.tensor.matmul(
                        psuv,
                        AP(x1H, 2 * kt, [[2 * KT, 128], [1, 2]]).bitcast(f32r),
                        AP(ptH, kt * 512, [[KT * 512, 128], [1, 512]]).bitcast(f32r),
                        start=(kt == 0),
                        stop=False,
                    )
                for kt in range(KT):
                    nc.tensor.matmul(
                        psuv,
                        AP(w2H, 2 * kt, [[2 * KT, 128], [1, 2]]).bitcast(f32r),
                        AP(qtH, kt * 512, [[KT * 512, 128], [1, 512]]).bitcast(f32r),
                        start=False,
                        stop=(kt == KT - 1),
                    )
                xssb = small.tile([2, 512], f32, name=f"xs_{t}", tag="xssb")
                nc.scalar.activation(xssb, psuv, AF.Tanh)

                if t == STEPS - 1:
                    # Final step: no core needs the gathered state again.
                    # Each core writes its own slice into out[32,:,0:512];
                    # the host reassembles across cores.
                    nc.sync.dma_start(
                        AP(out, STEPS * 2 * N, [[N, 2], [1, 512]]), xssb
                    )
                    continue

                # Keep the PE HAM-warm through the AllGather gap: dummy
                # matmuls on resident data into a write-only PSUM bank.
                if t + 1 < STEPS:
                    pswm = psB.tile(
                        [2, 512], f32, name=f"warm_{t}", tag="warm", bufs=1
                    )
                    for dk in range(N_WARM):
                        nc.tensor.matmul(
                            pswm,
                            AP(x1H, 2 * (dk % KT),
                               [[2 * KT, 128], [1, 2]]).bitcast(f32r),
                            AP(ptH, (dk % KT) * 512,
                               [[KT * 512, 128], [1, 512]]).bitcast(f32r),
                            start=(dk == 0),
                            stop=(dk == N_WARM - 1),
                        )

                # state slice -> DRAM bounce, AllGather, distribute
                agin = dpool.tile([2, 512], f32, name=f"agin_{t}", tag="agin")
                nc.sync.dma_start(agin, xssb)
                agout = dpool.tile(
                    [NCORES, 2, 512], f32, name=f"agout_{t}", tag="agout",
                    addr_space="Shared",
                )
                nc.gpsimd.collective_compute(
                    "AllGather",
                    mybir.AluOpType.bypass,
                    replica_groups=[list(range(NCORES))],
                    ins=[agin],
                    outs=[agout],
                )
                agoH = agout.tensor
                # out[t+1, c, r*512+nl] <- agout[r, c, nl]
                nc.gpsimd.dma_start(
                    AP(out, (t + 1) * 2 * N, [[512, 8], [N, 2], [1, 512]]),
                    AP(agoH, 0, [[1024, 8], [512, 2], [1, 512]]),
                )
                if t + 1 < STEPS:
                    # agout [8, 2, 512] flat-contiguous matches xa [16, 512]
                    x1, w2 = load_xw(
                        t + 1, AP(agoH, 0, [[512, 16], [1, 512]])
                    )

    nc.compile()
    return nc


def _get_nc():
    if "nc" not in _CACHE:
        _CACHE["nc"] = _build_nc()
    return _CACHE["nc"]


def run(inputs, trace=False):
    from concourse import bass_utils

    nc = _get_nc()
    x = np.asarray(inputs["x"], np.float32)
    xT = np.ascontiguousarray(x.T)  # [2, N]
    om = float(np.asarray(inputs["omega"], np.float32))
    ts = np.arange(STEPS, dtype=np.float32) * np.float32(om)
    c, s = np.cos(ts, dtype=np.float32), np.sin(ts, dtype=np.float32)
    row = np.zeros(2 * STEPS, np.float32)
    row[0::2] = c
    row[1::2] = s
    wrot = np.ascontiguousarray(np.broadcast_to(row, (128, 2 * STEPS)))
    mats = {
        "s_sl": np.asarray(inputs["raw_S"], np.float32),
        "ph_sl": np.asarray(inputs["raw_phase"], np.float32),
        "r_sl": np.asarray(inputs["raw_r"], np.float32),
        "m_sl": np.asarray(inputs["A_mask"], np.float32),
        "g_sl": np.asarray(inputs["G_gate"], np.float32),
    }
    in_maps = []
    for c in range(NCORES):
        rows = slice(c * ROWS, (c + 1) * ROWS)
        im = {k: np.ascontiguousarray(v[rows]) for k, v in mats.items()}
        im["xfull"] = xT
        im["wrot"] = wrot
        in_maps.append(im)
    res = bass_utils.run_bass_kernel_spmd(
        nc, in_maps, core_ids=list(range(NCORES)), trace=trace
    )
    out = np.array(res.results[0]["out"], np.float32, copy=True)  # [33, 2, N]
    # final step skips the AllGather: core i wrote its slice to
    # out[32, :, 0:512]; reassemble the full last row across cores
    out[STEPS] = np.concatenate(
        [np.asarray(res.results[i]["out"], np.float32)[STEPS, :, 0:ROWS]
         for i in range(NCORES)],
        axis=1,
    )
    full = np.ascontiguousarray(out.transpose(0, 2, 1))  # [33, N, 2]
    return full, res


def kernel(**inputs):
    full, _ = run(inputs, trace=False)
    return full



# revision 9
# speedup vs baseline: 1.3000x; 1.3000x over previous
"""Trainium2 Bass kernel for CausalCoreV5 (complex-weight GNN message passing).

Math: reference does, per step t:
    theta = raw_phase + omega*t ;  c,s = cos(theta), sin(theta)
    Aamp  = A_mask * G_gate * tanh(raw_S) * sigmoid(raw_r)
    out_r = (Aamp*c)@xr - (Aamp*s)@xi ;  out_i = (Aamp*s)@xr + (Aamp*c)@xi
    x'    = tanh([out_r, out_i])

Angle-addition turns the time-varying matrices into two FIXED matrices:
    P = Aamp*cos(raw_phase), Q = Aamp*sin(raw_phase)
    u = P@xr - Q@xi ; v = Q@xr + P@xi
    out_r = cos(wt)*u - sin(wt)*v ; out_i = sin(wt)*u + cos(wt)*v

Each of 8 cores owns 512 output rows. P^T,Q^T slices (4096x512) are stored
in SBUF as float8e4 (e4m3) scaled by 64; host pre-transposes the five input
slices so the load phase is pure DMA + elementwise (no PE transposes).
Steady state: 32 DoubleRow fp8 matmuls per step (k-tile pairs fused, 2x PE
rate), tanh(PSUM/2048) -> f32 state slice; the slice is scaled by 32, cast
to fp8 and AllGathered (1KB payload); receivers PE-transpose the gathered
state and fold the per-step rotation (scaled by 32) into the x1/w2 weights.
Cheap DoubleRow warm matmuls keep the PE busy across the AllGather gap.
"""

import os
import sys

import numpy as np

if "/opt/trn_rl_repo" not in sys.path:
    sys.path.insert(0, "/opt/trn_rl_repo")

N = 4096
STEPS = 32
NCORES = 8
ROWS = N // NCORES          # 512 output rows per core
KT = N // 128               # 32 contraction k-tiles of 128
NG = KT // 2                # 16 DoubleRow groups (k-tile pairs)
N_WARM = 24                 # warm-keeper matmuls per AllGather gap
CHUNK_KT = 4                # load-chunk k-tiles -> [128, 2048] f32 tiles
SCALE_PQ = 64.0             # fp8 scale on P,Q
SCALE_X = 32.0              # fp8 scale on state
INV_SCALE = 1.0 / (SCALE_PQ * SCALE_X)

_CACHE = {}


def _build_nc():
    import math

    from concourse import bacc, bass, masks, mybir, tile
    from concourse.bass import AP

    f32 = mybir.dt.float32
    fp8 = mybir.dt.float8e4
    AF = mybir.ActivationFunctionType
    DR = mybir.MatmulPerfMode.DoubleRow
    HALF_PI = math.pi / 2.0

    nc = bacc.Bacc(
        "TRN2",
        target_bir_lowering=False,
        debug=False,
        enable_asserts=True,
        num_devices=NCORES,
    )

    # Register pi/2 as a const AP (used as Sin bias to get cos).
    _hp = nc.alloc_sbuf_tensor("const-halfpi", [128, 1], f32)
    nc.gpsimd.memset(_hp.ap(), HALF_PI)
    nc.const_aps.aps[(f32, HALF_PI)] = _hp.ap()
    nc.all_engine_barrier()

    # xfull comes in TRANSPOSED: [2, N] (xr row, xi row).
    xfull = nc.dram_tensor("xfull", [2, N], f32, kind="ExternalInput")
    # Pre-transposed per-core slices: mt_*[k, m] = raw[512*core + m, k].
    mt_s = nc.dram_tensor("mt_s", [N, ROWS], f32, kind="ExternalInput")
    mt_p = nc.dram_tensor("mt_p", [N, ROWS], f32, kind="ExternalInput")
    mt_r = nc.dram_tensor("mt_r", [N, ROWS], f32, kind="ExternalInput")
    mt_m = nc.dram_tensor("mt_m", [N, ROWS], f32, kind="ExternalInput")
    mt_g = nc.dram_tensor("mt_g", [N, ROWS], f32, kind="ExternalInput")
    # Per-step rotation scalars 32*cos(wt), 32*sin(wt) broadcast down 128
    # partitions (cols 2t, 2t+1); derived on host from the scalar omega.
    wrot = nc.dram_tensor("wrot", [128, 2 * STEPS], f32, kind="ExternalInput")
    # Each core writes only its own [2, 512] slice per step; host reassembles.
    out = nc.dram_tensor("out", [STEPS + 1, 2, ROWS], f32, kind="ExternalOutput")

    with tile.TileContext(nc) as tc:
        with (
            tc.tile_pool(name="big", bufs=1) as big,
            tc.tile_pool(name="work", bufs=2) as work,
            tc.tile_pool(name="small", bufs=2) as small,
            tc.tile_pool(name="psA", bufs=2, space="PSUM") as psA,
            tc.tile_pool(name="psB", bufs=2, space="PSUM") as psB,
            tc.tile_pool(name="dram", bufs=2, space="DRAM") as dpool,
        ):
            identf = big.tile([16, 16], f32, name="identf", tag="identf")
            masks.make_identity(nc, identf)
            ident8 = big.tile([16, 16], fp8, name="ident8", tag="ident8")
            masks.make_identity(nc, ident8)

            # Persistent transposed matrices: PT[k, n], QT[k, n] as 32 k-tiles
            # of [128, 512] side by side -> [128, 32*512] in fp8 (x64 scale).
            pt = big.tile([128, KT * 512], fp8, name="pt", tag="pt")
            qt = big.tile([128, KT * 512], fp8, name="qt", tag="qt")
            ptH = pt.tensor
            qtH = qt.tensor

            # per-step rotation scalars (x32)
            W = 2 * STEPS
            wrs = small.tile([128, W], f32, name="wrs", tag="wrs", bufs=1)
            nc.sync.dma_start(wrs, wrot[0:128, 0:W])
            wrsH = wrs.tensor

            def prep_from_psx(t, psx, x1, w2, estep=1):
                """psx [128,64*estep] PSUM with psx[p, estep*(16j+2r+c)] =
                x[c, (4r+j)*128+p] (fp8 transpose writes element step 2).
                Builds x1[p, 2kt+c] = rot_t(x)*32 in fp8 and w2 = [-xi'|xr']."""
                pxH = psx.tensor
                x1H, w2H = x1.tensor, w2.tensor
                c_t = AP(wrsH, 2 * t, [[W, 128], [1, 1]])
                s_t = AP(wrsH, 2 * t + 1, [[W, 128], [1, 1]])
                e = estep
                xr_ap = AP(pxH, 0, [[64 * e, 128], [16 * e, 4], [2 * e, 8]])
                xi_ap = AP(pxH, e, [[64 * e, 128], [16 * e, 4], [2 * e, 8]])
                tA = small.tile([128, KT], f32, name=f"tA_{t}", tag="tA")
                tB = small.tile([128, KT], f32, name=f"tB_{t}", tag="tB")
                tC = small.tile([128, KT], f32, name=f"tC_{t}", tag="tC")
                tD = small.tile([128, KT], f32, name=f"tD_{t}", tag="tD")
                # tA/tB must be kt-ordered: kt = 4r + j for iter dims (j, r)
                t3 = [[KT, 128], [1, 4], [4, 8]]
                x1e = AP(x1H, 0, [[2 * KT, 128], [2, KT]])
                x1o = AP(x1H, 1, [[2 * KT, 128], [2, KT]])
                w2e = AP(w2H, 0, [[2 * KT, 128], [2, KT]])
                w2o = AP(w2H, 1, [[2 * KT, 128], [2, KT]])
                # Critical chain first: x1 even cols (xr') gate the PT matmuls.
                nc.vector.tensor_scalar_mul(AP(tA.tensor, 0, t3), xr_ap, c_t)
                nc.vector.tensor_scalar_mul(AP(tB.tensor, 0, t3), xi_ap, s_t)
                nc.vector.tensor_tensor(
                    x1e, tA, tB, op=mybir.AluOpType.subtract
                )
                nc.vector.tensor_scalar_mul(AP(tC.tensor, 0, t3), xr_ap, s_t)
                nc.vector.tensor_scalar_mul(AP(tD.tensor, 0, t3), xi_ap, c_t)
                nc.vector.tensor_tensor(
                    x1o, tC, tD, op=mybir.AluOpType.add
                )
                nc.vector.tensor_scalar_mul(w2e, x1o, -1.0)
                nc.vector.tensor_copy(w2o, x1e)

            # ---------------- initial state -> x1/w2 (before load loop so its
            # DMA + transposes run during the load) ------------------------
            xa0 = work.tile([16, 512], f32, name="xa0", tag="xa0")
            nc.sync.dma_start(xa0, AP(xfull, 0, [[512, 8], [N, 2], [1, 512]]))
            x1 = small.tile([128, 2 * KT], fp8, name="x1_0", tag="x1")
            w2 = small.tile([128, 2 * KT], fp8, name="w2_0", tag="w2")
            psx0 = psA.tile([128, 64], f32, name="psx0", tag="psx")
            for j in range(4):
                nc.tensor.transpose(
                    psx0[:, 16 * j:16 * (j + 1)],
                    xa0[:, j * 128:(j + 1) * 128],
                    identf,
                )
            prep_from_psx(0, psx0, x1, w2)

            # ---------------- Phase A: load inputs, build P^T, Q^T ----------
            CC = CHUNK_KT * 512  # 2048 columns per chunk
            for c8 in range(KT // CHUNK_KT):  # 8 chunks of 4 k-tiles
                src3 = [[ROWS, 128], [128 * ROWS, CHUNK_KT], [1, ROWS]]
                off = c8 * CHUNK_KT * 128 * ROWS
                s_in = work.tile([128, CC], f32, name=f"s_{c8}", tag="s_in")
                r_in = work.tile([128, CC], f32, name=f"r_{c8}", tag="r_in")
                m_in = work.tile([128, CC], f32, name=f"m_{c8}", tag="m_in")
                g_in = work.tile([128, CC], f32, name=f"g_{c8}", tag="g_in")
                p_in = work.tile([128, CC], f32, name=f"p_{c8}", tag="p_in")
                nc.sync.dma_start(s_in, AP(mt_s, off, src3))
                nc.sync.dma_start(m_in, AP(mt_m, off, src3))
                nc.gpsimd.dma_start(p_in, AP(mt_p, off, src3))
                nc.gpsimd.dma_start(g_in, AP(mt_g, off, src3))
                nc.scalar.dma_start(r_in, AP(mt_r, off, src3))

                cos_t = work.tile([128, CC], f32, name=f"c_{c8}", tag="cos_t")
                sin_t = work.tile([128, CC], f32, name=f"n_{c8}", tag="sin_t")

                # sigmoid via tanh keeps ACT on two LUTs; alternate emission
                # order per chunk parity so ACT reloads each LUT once/chunk.
                def _tanh_ops():
                    nc.scalar.activation(s_in, s_in, AF.Tanh)
                    nc.scalar.activation(r_in, r_in, AF.Tanh, scale=0.5)

                def _sin_ops():
                    nc.scalar.activation(cos_t, p_in, AF.Sin, bias=HALF_PI)
                    nc.scalar.activation(sin_t, p_in, AF.Sin)

                if c8 % 2 == 0:
                    _tanh_ops(); _sin_ops()
                else:
                    _sin_ops(); _tanh_ops()

                # r_in <- 64*sigmoid(raw_r) = 32*tanh(raw_r/2) + 32
                nc.vector.tensor_scalar(
                    r_in, r_in, SCALE_X, SCALE_X,
                    op0=mybir.AluOpType.mult, op1=mybir.AluOpType.add,
                )
                nc.gpsimd.tensor_mul(m_in, m_in, g_in)
                nc.gpsimd.tensor_mul(m_in, m_in, s_in)
                nc.vector.tensor_mul(m_in, m_in, r_in)  # 64*Aamp
                pdst = AP(ptH, c8 * CC, [[KT * 512, 128], [1, CC]])
                qdst = AP(qtH, c8 * CC, [[KT * 512, 128], [1, CC]])
                nc.vector.tensor_mul(pdst, cos_t, m_in)   # fp8 cast on write
                nc.vector.tensor_mul(qdst, sin_t, m_in)

            # ---------------- time loop -------------------------------------
            for t in range(STEPS):
                psuv = psB.tile([2, 512], f32, name=f"uv_{t}", tag="uv")
                x1H, w2H = x1.tensor, w2.tensor
                # DoubleRow fuses k-tiles (g, g+16): weight plane stride 32
                # (must be %16==0), moving plane stride 16*512.
                for g in range(NG):
                    nc.tensor.matmul(
                        psuv,
                        AP(x1H, 2 * g, [[2 * KT, 128], [32, 2], [1, 2]]),
                        AP(ptH, g * 512,
                           [[KT * 512, 128], [NG * 512, 2], [1, 512]]),
                        start=(g == 0),
                        stop=False,
                        perf_mode=DR,
                    )
                for g in range(NG):
                    nc.tensor.matmul(
                        psuv,
                        AP(w2H, 2 * g, [[2 * KT, 128], [32, 2], [1, 2]]),
                        AP(qtH, g * 512,
                           [[KT * 512, 128], [NG * 512, 2], [1, 512]]),
                        start=False,
                        stop=(g == NG - 1),
                        perf_mode=DR,
                    )
                xssb = small.tile([2, 512], f32, name=f"xs_{t}", tag="xssb")
                nc.scalar.activation(xssb, psuv, AF.Tanh, scale=INV_SCALE)
                # trajectory: own slice only; host reassembles across cores
                nc.gpsimd.dma_start(
                    AP(out, (t + 1) * 2 * ROWS, [[ROWS, 2], [1, ROWS]]), xssb
                )
                if t == STEPS - 1:
                    continue

                # state slice *32 -> fp8 -> DRAM bounce -> AllGather
                xsend = small.tile([2, 512], fp8, name=f"xf_{t}", tag="xsend")
                nc.vector.tensor_scalar_mul(xsend, xssb, SCALE_X)
                agin = dpool.tile([2, 512], fp8, name=f"agin_{t}", tag="agin")
                nc.sync.dma_start(agin, xsend)
                agout = dpool.tile(
                    [NCORES, 2, 512], fp8, name=f"agout_{t}", tag="agout",
                    addr_space="Shared",
                )
                nc.gpsimd.collective_compute(
                    "AllGather",
                    mybir.AluOpType.bypass,
                    replica_groups=[list(range(NCORES))],
                    ins=[agin],
                    outs=[agout],
                )

                # Keep the PE busy through the AllGather gap: cheap DoubleRow
                # matmuls on resident data into a write-only PSUM bank.
                pswm = psB.tile(
                    [2, 512], f32, name=f"warm_{t}", tag="warm", bufs=1
                )
                for dk in range(N_WARM):
                    g = dk % NG
                    nc.tensor.matmul(
                        pswm,
                        AP(x1H, 2 * g, [[2 * KT, 128], [32, 2], [1, 2]]),
                        AP(ptH, g * 512,
                           [[KT * 512, 128], [NG * 512, 2], [1, 512]]),
                        start=(dk == 0),
                        stop=(dk == N_WARM - 1),
                        perf_mode=DR,
                    )

                # gathered fp8 state -> weight layout for step t+1
                agoH = agout.tensor
                xa = work.tile([16, 512], fp8, name=f"xa_{t}", tag="xa")
                nc.sync.dma_start(xa, AP(agoH, 0, [[512, 16], [1, 512]]))
                x1 = small.tile([128, 2 * KT], fp8, name=f"x1_{t+1}", tag="x1")
                w2 = small.tile([128, 2 * KT], fp8, name=f"w2_{t+1}", tag="w2")
                psx = psA.tile([128, 128], fp8, name=f"px_{t}", tag="psx8")
                pxH8 = psx.tensor
                for j in range(4):
                    nc.tensor.transpose(
                        AP(pxH8, 32 * j, [[128, 128], [2, 16]]),
                        xa[:, j * 128:(j + 1) * 128],
                        ident8,
                    )
                prep_from_psx(t + 1, psx, x1, w2, estep=2)

    nc.compile()
    return nc


def _get_nc():
    if "nc" not in _CACHE:
        _CACHE["nc"] = _build_nc()
    return _CACHE["nc"]


def run(inputs, trace=False):
    from concourse import bass_utils

    nc = _get_nc()
    x = np.asarray(inputs["x"], np.float32)
    xT = np.ascontiguousarray(x.T)  # [2, N]
    om = float(np.asarray(inputs["omega"], np.float32))
    ts = np.arange(STEPS, dtype=np.float32) * np.float32(om)
    c, s = np.cos(ts, dtype=np.float32), np.sin(ts, dtype=np.float32)
    # t=0 prep consumes the unscaled f32 input state (needs x32 here);
    # t>=1 preps consume the AllGathered state already scaled by 32.
    row = np.zeros(2 * STEPS, np.float32)
    row[0::2] = c
    row[1::2] = s
    row[0:2] *= np.float32(SCALE_X)
    wrot = np.ascontiguousarray(np.broadcast_to(row, (128, 2 * STEPS)))
    mats = {
        "mt_s": np.asarray(inputs["raw_S"], np.float32),
        "mt_p": np.asarray(inputs["raw_phase"], np.float32),
        "mt_r": np.asarray(inputs["raw_r"], np.float32),
        "mt_m": np.asarray(inputs["A_mask"], np.float32),
        "mt_g": np.asarray(inputs["G_gate"], np.float32),
    }
    in_maps = []
    for ci in range(NCORES):
        rows = slice(ci * ROWS, (ci + 1) * ROWS)
        im = {k: np.ascontiguousarray(v[rows].T) for k, v in mats.items()}
        im["xfull"] = xT
        im["wrot"] = wrot
        in_maps.append(im)
    res = bass_utils.run_bass_kernel_spmd(
        nc, in_maps, core_ids=list(range(NCORES)), trace=trace
    )
    # reassemble: core i owns output columns [512*i, 512*(i+1))
    full = np.empty((STEPS + 1, 2, N), np.float32)
    full[0] = xT
    for i in range(NCORES):
        oi = np.asarray(res.results[i]["out"], np.float32)  # [33, 2, 512]
        full[1:, :, i * ROWS:(i + 1) * ROWS] = oi[1:]
    return np.ascontiguousarray(full.transpose(0, 2, 1)), res


def kernel(**inputs):
    full, _ = run(inputs, trace=False)
    return full


# revision 10
# speedup vs baseline: 1.4035x; 1.0796x over previous
"""Trainium2 Bass kernel for CausalCoreV5 (complex-weight GNN message passing).

Math: reference does, per step t:
    theta = raw_phase + omega*t ;  c,s = cos(theta), sin(theta)
    Aamp  = A_mask * G_gate * tanh(raw_S) * sigmoid(raw_r)
    out_r = (Aamp*c)@xr - (Aamp*s)@xi ;  out_i = (Aamp*s)@xr + (Aamp*c)@xi
    x'    = tanh([out_r, out_i])

Angle-addition turns the time-varying matrices into two FIXED matrices:
    P = Aamp*cos(raw_phase), Q = Aamp*sin(raw_phase)
    u = P@xr - Q@xi ; v = Q@xr + P@xi
    out_r = cos(wt)*u - sin(wt)*v ; out_i = sin(wt)*u + cos(wt)*v

Each of 8 cores owns 512 output rows. The host stages its five input slices
pre-transposed, partition-major, in bf16 ([128, kt, m] so DMA bursts are 4KB
contiguous); the load phase is pure DMA + elementwise and builds P^T/Q^T in
SBUF as float8e4 scaled by 64. Steady state: 32 DoubleRow fp8 matmuls per
step (adjacent k-tile pairs fused -> 2x PE rate; adjacency also lets step 0
stream behind the load), tanh(PSUM/2048) in bf16, bf16 state AllGather (2KB),
PE-transpose of the gathered state, and the per-step rotation (x32, for fp8
range) folded into fp8 x1/w2 weights. The w2 prep overlaps the P matmuls;
cheap DoubleRow warm matmuls keep the PE busy across the AllGather gap.
"""

import os
import sys

import numpy as np

if "/opt/trn_rl_repo" not in sys.path:
    sys.path.insert(0, "/opt/trn_rl_repo")

N = 4096
STEPS = 32
NCORES = 8
ROWS = N // NCORES          # 512 output rows per core
KT = N // 128               # 32 contraction k-tiles of 128
NG = KT // 2                # 16 DoubleRow groups (adjacent k-tile pairs)
N_WARM = 24                 # warm-keeper matmuls per AllGather gap
CHUNK_KT = 4                # load-chunk k-tiles -> [128, 2048] tiles
SCALE_PQ = 64.0             # fp8 scale on P,Q
SCALE_X = 32.0              # fp8 scale on the rotated state weights
INV_SCALE = 1.0 / (SCALE_PQ * SCALE_X)

_CACHE = {}


def _build_nc():
    import math

    from concourse import bacc, bass, masks, mybir, tile
    from concourse.bass import AP

    f32 = mybir.dt.float32
    bf16 = mybir.dt.bfloat16
    fp8 = mybir.dt.float8e4
    AF = mybir.ActivationFunctionType
    DR = mybir.MatmulPerfMode.DoubleRow
    HALF_PI = math.pi / 2.0

    nc = bacc.Bacc(
        "TRN2",
        target_bir_lowering=False,
        debug=False,
        enable_asserts=True,
        num_devices=NCORES,
    )

    # Register pi/2 as a const AP (used as Sin bias to get cos).
    _hp = nc.alloc_sbuf_tensor("const-halfpi", [128, 1], f32)
    nc.gpsimd.memset(_hp.ap(), HALF_PI)
    nc.const_aps.aps[(f32, HALF_PI)] = _hp.ap()
    nc.all_engine_barrier()

    # xfull comes in TRANSPOSED: [2, N] (xr row, xi row).
    xfull = nc.dram_tensor("xfull", [2, N], f32, kind="ExternalInput")
    # Pre-transposed, partition-major bf16 slices:
    # mt_*[p, kt*512 + m] = raw[512*core + m, kt*128 + p].
    mt_s = nc.dram_tensor("mt_s", [128, KT * 512], bf16, kind="ExternalInput")
    mt_p = nc.dram_tensor("mt_p", [128, KT * 512], bf16, kind="ExternalInput")
    mt_r = nc.dram_tensor("mt_r", [128, KT * 512], bf16, kind="ExternalInput")
    mt_m = nc.dram_tensor("mt_m", [128, KT * 512], bf16, kind="ExternalInput")
    mt_g = nc.dram_tensor("mt_g", [128, KT * 512], bf16, kind="ExternalInput")
    # Per-step rotation scalars 32*cos(wt), 32*sin(wt) broadcast down 128
    # partitions (cols 2t, 2t+1); derived on host from the scalar omega.
    wrot = nc.dram_tensor("wrot", [128, 2 * STEPS], f32, kind="ExternalInput")
    # Each core writes only its own [2, 512] slice per step; host reassembles.
    out = nc.dram_tensor("out", [STEPS + 1, 2, ROWS], bf16, kind="ExternalOutput")

    with tile.TileContext(nc) as tc:
        with (
            tc.tile_pool(name="big", bufs=1) as big,
            tc.tile_pool(name="work", bufs=2) as work,
            tc.tile_pool(name="small", bufs=2) as small,
            tc.tile_pool(name="psA", bufs=2, space="PSUM") as psA,
            tc.tile_pool(name="psB", bufs=2, space="PSUM") as psB,
            tc.tile_pool(name="dram", bufs=2, space="DRAM") as dpool,
        ):
            identf = big.tile([16, 16], f32, name="identf", tag="identf")
            masks.make_identity(nc, identf)
            identb = big.tile([16, 16], bf16, name="identb", tag="identb")
            masks.make_identity(nc, identb)

            # Persistent transposed matrices: PT[k, n], QT[k, n] as 32 k-tiles
            # of [128, 512] side by side -> [128, 32*512] in fp8 (x64 scale).
            pt = big.tile([128, KT * 512], fp8, name="pt", tag="pt")
            qt = big.tile([128, KT * 512], fp8, name="qt", tag="qt")
            ptH = pt.tensor
            qtH = qt.tensor

            # per-step rotation scalars (x32)
            W = 2 * STEPS
            wrs = small.tile([128, W], f32, name="wrs", tag="wrs", bufs=1)
            nc.sync.dma_start(wrs, wrot[0:128, 0:W])
            wrsH = wrs.tensor

            # x1/w2 column layout (DoubleRow plane step must be %16==0):
            # col(kt, c) = 32*(kt&1) + 2*(kt>>1) + c, so the adjacent pair
            # (2k, 2k+1) has planes at cols {2k, 2k+32} (stride 32).
            def prep_x1(t, psx, x1, e):
                """psx [128,64*e] with psx[p, e*(16j+2r+c)] = x[c,(4r+j)*128+p].
                x1[p, col] = rot_t(x)*32 in fp8 (x1e then x1o)."""
                pxH = psx.tensor
                x1H = x1.tensor
                c_t = AP(wrsH, 2 * t, [[W, 128], [1, 1]])
                s_t = AP(wrsH, 2 * t + 1, [[W, 128], [1, 1]])
                xr_ap = AP(pxH, 0, [[64 * e, 128], [16 * e, 4], [2 * e, 8]])
                xi_ap = AP(pxH, e, [[64 * e, 128], [16 * e, 4], [2 * e, 8]])
                tA = small.tile([128, KT], f32, name=f"tA_{t}", tag="tA")
                tB = small.tile([128, KT], f32, name=f"tB_{t}", tag="tB")
                tC = small.tile([128, KT], f32, name=f"tC_{t}", tag="tC")
                tD = small.tile([128, KT], f32, name=f"tD_{t}", tag="tD")
                # tA..tD are kt-ordered: kt = 4r + j for iter dims (j, r)
                t3 = [[KT, 128], [1, 4], [4, 8]]
                # dst iterates kt = 2a+b ascending -> col = 2a + 32b (+base)
                src2 = [[KT, 128], [2, 16], [1, 2]]
                dst2 = [[2 * KT, 128], [2, 16], [32, 2]]
                nc.vector.tensor_scalar_mul(AP(tA.tensor, 0, t3), xr_ap, c_t)
                nc.vector.tensor_scalar_mul(AP(tB.tensor, 0, t3), xi_ap, s_t)
                nc.vector.tensor_tensor(
                    AP(x1H, 0, dst2), AP(tA.tensor, 0, src2),
                    AP(tB.tensor, 0, src2), op=mybir.AluOpType.subtract,
                )
                nc.vector.tensor_scalar_mul(AP(tC.tensor, 0, t3), xr_ap, s_t)
                nc.vector.tensor_scalar_mul(AP(tD.tensor, 0, t3), xi_ap, c_t)
                nc.vector.tensor_tensor(
                    AP(x1H, 1, dst2), AP(tC.tensor, 0, src2),
                    AP(tD.tensor, 0, src2), op=mybir.AluOpType.add,
                )

            def prep_w2(t, x1, w2):
                """w2 = [-xi'|xr'] from x1 = [xr'|xi'] (pairwise col swap)."""
                x1H, w2H = x1.tensor, w2.tensor
                flat = [[2 * KT, 128], [2, KT]]
                nc.vector.tensor_scalar_mul(
                    AP(w2H, 0, flat), AP(x1H, 1, flat), -1.0
                )
                nc.vector.tensor_copy(AP(w2H, 1, flat), AP(x1H, 0, flat))

            # ---------------- initial state -> x1/w2 (before load loop so its
            # DMA + transposes run during the load) ------------------------
            xa0 = work.tile([16, 512], f32, name="xa0", tag="xa0")
            nc.sync.dma_start(xa0, AP(xfull, 0, [[512, 8], [N, 2], [1, 512]]))
            x1 = small.tile([128, 2 * KT], fp8, name="x1_0", tag="x1")
            w2 = small.tile([128, 2 * KT], fp8, name="w2_0", tag="w2")
            psx0 = psA.tile([128, 64], f32, name="psx0", tag="psx")
            for j in range(4):
                nc.tensor.transpose(
                    psx0[:, 16 * j:16 * (j + 1)],
                    xa0[:, j * 128:(j + 1) * 128],
                    identf,
                )
            prep_x1(0, psx0, x1, 1)
            prep_w2(0, x1, w2)

            # ---------------- Phase A: load inputs, build P^T, Q^T ----------
            CC = CHUNK_KT * 512  # 2048 columns per chunk
            for c8 in range(KT // CHUNK_KT):  # 8 chunks of 4 k-tiles
                src = [[KT * 512, 128], [1, CC]]
                off = c8 * CC
                s_in = work.tile([128, CC], bf16, name=f"s_{c8}", tag="s_in")
                r_in = work.tile([128, CC], bf16, name=f"r_{c8}", tag="r_in")
                m_in = work.tile([128, CC], bf16, name=f"m_{c8}", tag="m_in")
                g_in = work.tile([128, CC], bf16, name=f"g_{c8}", tag="g_in")
                p_in = work.tile([128, CC], bf16, name=f"p_{c8}", tag="p_in")
                nc.sync.dma_start(s_in, AP(mt_s, off, src))
                nc.sync.dma_start(m_in, AP(mt_m, off, src))
                nc.scalar.dma_start(r_in, AP(mt_r, off, src))
                nc.scalar.dma_start(g_in, AP(mt_g, off, src))
                nc.gpsimd.dma_start(p_in, AP(mt_p, off, src))

                cos_t = work.tile([128, CC], bf16, name=f"c_{c8}", tag="cos_t")
                sin_t = work.tile([128, CC], bf16, name=f"n_{c8}", tag="sin_t")

                # sigmoid via tanh keeps ACT on two LUTs; alternate emission
                # order per chunk parity so ACT reloads each LUT once/chunk.
                def _tanh_ops():
                    nc.scalar.activation(s_in, s_in, AF.Tanh)
                    nc.scalar.activation(r_in, r_in, AF.Tanh, scale=0.5)

                def _sin_ops():
                    nc.scalar.activation(cos_t, p_in, AF.Sin, bias=HALF_PI)
                    nc.scalar.activation(sin_t, p_in, AF.Sin)

                if c8 % 2 == 0:
                    _tanh_ops(); _sin_ops()
                else:
                    _sin_ops(); _tanh_ops()

                # r_in <- 64*sigmoid(raw_r) = 32*tanh(raw_r/2) + 32
                nc.vector.tensor_scalar(
                    r_in, r_in, SCALE_X, SCALE_X,
                    op0=mybir.AluOpType.mult, op1=mybir.AluOpType.add,
                )
                nc.vector.tensor_mul(m_in, m_in, g_in)
                nc.gpsimd.tensor_mul(m_in, m_in, s_in)
                nc.vector.tensor_mul(m_in, m_in, r_in)  # 64*Aamp
                pdst = AP(ptH, c8 * CC, [[KT * 512, 128], [1, CC]])
                qdst = AP(qtH, c8 * CC, [[KT * 512, 128], [1, CC]])
                nc.vector.tensor_mul(pdst, cos_t, m_in)   # fp8 cast on write
                nc.vector.tensor_mul(qdst, sin_t, m_in)

            # ---------------- time loop -------------------------------------
            for t in range(STEPS):
                psuv = psB.tile([2, 512], f32, name=f"uv_{t}", tag="uv")
                x1H, w2H = x1.tensor, w2.tensor
                # DoubleRow fuses adjacent k-tiles (2k, 2k+1): weight planes
                # at cols {2k, 2k+32} (stride 32), moving planes adjacent.
                for g in range(NG):
                    nc.tensor.matmul(
                        psuv,
                        AP(x1H, 2 * g, [[2 * KT, 128], [32, 2], [1, 2]]),
                        AP(ptH, g * 1024,
                           [[KT * 512, 128], [512, 2], [1, 512]]),
                        start=(g == 0),
                        stop=False,
                        perf_mode=DR,
                    )
                for g in range(NG):
                    nc.tensor.matmul(
                        psuv,
                        AP(w2H, 2 * g, [[2 * KT, 128], [32, 2], [1, 2]]),
                        AP(qtH, g * 1024,
                           [[KT * 512, 128], [512, 2], [1, 512]]),
                        start=False,
                        stop=(g == NG - 1),
                        perf_mode=DR,
                    )
                xssb = small.tile([2, 512], bf16, name=f"xs_{t}", tag="xssb")
                nc.scalar.activation(xssb, psuv, AF.Tanh, scale=INV_SCALE)
                # trajectory: own slice only; host reassembles across cores
                nc.gpsimd.dma_start(
                    AP(out, (t + 1) * 2 * ROWS, [[ROWS, 2], [1, ROWS]]), xssb
                )
                if t == STEPS - 1:
                    continue

                # bf16 state slice -> DRAM bounce -> AllGather
                agin = dpool.tile([2, 512], bf16, name=f"agin_{t}", tag="agin")
                nc.sync.dma_start(agin, xssb)
                agout = dpool.tile(
                    [NCORES, 2, 512], bf16, name=f"agout_{t}", tag="agout",
                    addr_space="Shared",
                )
                nc.gpsimd.collective_compute(
                    "AllGather",
                    mybir.AluOpType.bypass,
                    replica_groups=[list(range(NCORES))],
                    ins=[agin],
                    outs=[agout],
                )

                # Keep the PE busy through the AllGather gap: cheap DoubleRow
                # matmuls on resident data into a write-only PSUM bank.
                pswm = psB.tile(
                    [2, 512], f32, name=f"warm_{t}", tag="warm", bufs=1
                )
                for dk in range(N_WARM):
                    g = dk % NG
                    nc.tensor.matmul(
                        pswm,
                        AP(x1H, 2 * g, [[2 * KT, 128], [32, 2], [1, 2]]),
                        AP(ptH, g * 1024,
                           [[KT * 512, 128], [512, 2], [1, 512]]),
                        start=(dk == 0),
                        stop=(dk == N_WARM - 1),
                        perf_mode=DR,
                    )

                # gathered bf16 state -> weight layout for step t+1
                agoH = agout.tensor
                xa = work.tile([16, 512], bf16, name=f"xa_{t}", tag="xa")
                nc.sync.dma_start(xa, AP(agoH, 0, [[512, 16], [1, 512]]))
                x1 = small.tile([128, 2 * KT], fp8, name=f"x1_{t+1}", tag="x1")
                w2 = small.tile([128, 2 * KT], fp8, name=f"w2_{t+1}", tag="w2")
                psx = psA.tile([128, 64], bf16, name=f"px_{t}", tag="psxb")
                for j in range(4):
                    nc.tensor.transpose(
                        psx[:, 16 * j:16 * (j + 1)],
                        xa[:, j * 128:(j + 1) * 128],
                        identb,
                    )
                prep_x1(t + 1, psx, x1, 1)
                # w2 prep overlaps the P matmuls of step t+1 (emitted next
                # loop iteration after the P block? no - emitted here, but
                # only the Q matmuls depend on it).
                prep_w2(t + 1, x1, w2)

    nc.compile()
    return nc


def _get_nc():
    if "nc" not in _CACHE:
        _CACHE["nc"] = _build_nc()
    return _CACHE["nc"]


def run(inputs, trace=False):
    import ml_dtypes

    from concourse import bass_utils

    nc = _get_nc()
    x = np.asarray(inputs["x"], np.float32)
    xT = np.ascontiguousarray(x.T)  # [2, N]
    om = float(np.asarray(inputs["omega"], np.float32))
    ts = np.arange(STEPS, dtype=np.float32) * np.float32(om)
    c, s = np.cos(ts, dtype=np.float32), np.sin(ts, dtype=np.float32)
    row = np.zeros(2 * STEPS, np.float32)
    row[0::2] = np.float32(SCALE_X) * c
    row[1::2] = np.float32(SCALE_X) * s
    wrot = np.ascontiguousarray(np.broadcast_to(row, (128, 2 * STEPS)))
    mats = {
        "mt_s": np.asarray(inputs["raw_S"], np.float32),
        "mt_p": np.asarray(inputs["raw_phase"], np.float32),
        "mt_r": np.asarray(inputs["raw_r"], np.float32),
        "mt_m": np.asarray(inputs["A_mask"], np.float32),
        "mt_g": np.asarray(inputs["G_gate"], np.float32),
    }
    in_maps = []
    for ci in range(NCORES):
        rows = slice(ci * ROWS, (ci + 1) * ROWS)
        im = {}
        for k, v in mats.items():
            # [512m, 4096k] -> T -> [32kt, 128p, 512m] -> [128, 32*512] bf16
            mt = v[rows].T.reshape(KT, 128, ROWS).transpose(1, 0, 2)
            im[k] = np.ascontiguousarray(
                mt.reshape(128, KT * ROWS).astype(ml_dtypes.bfloat16)
            )
        im["xfull"] = xT
        im["wrot"] = wrot
        in_maps.append(im)
    res = bass_utils.run_bass_kernel_spmd(
        nc, in_maps, core_ids=list(range(NCORES)), trace=trace
    )
    # reassemble: core i owns output columns [512*i, 512*(i+1))
    full = np.empty((STEPS + 1, 2, N), np.float32)
    full[0] = xT
    for i in range(NCORES):
        oi = np.asarray(res.results[i]["out"]).astype(np.float32)
        full[1:, :, i * ROWS:(i + 1) * ROWS] = oi[1:]
    return np.ascontiguousarray(full.transpose(0, 2, 1)), res


def kernel(**inputs):
    full, _ = run(inputs, trace=False)
    return full
